# revision 20
# baseline (speedup 1.0000x reference)
"""Banded chamfer distance (B=16, N=M=4096, D=3) on 8 Trainium2 NeuronCores.

Sharding: data-parallel over batch - 2 batches per core, SPMD.

Algorithm: both point clouds are sorted by the z coordinate on the host
(the output is permutation-invariant, so no un-sort is needed). After
sorting, the nearest neighbour of a point at sorted rank r is almost
always within a narrow window of ranks of the other cloud, because both
clouds are drawn from the same distribution. The device therefore only
computes a banded slice of the NxM squared-distance matrix: x row-tile t
(rows 128t..128t+127) against y columns OFFS[t]..OFFS[t]+W.

Exactness is restored on the host with a geometric certificate: for row n
the candidates excluded by the band all have |z - z_x[n]| at least the
z-gap to the band edge, so if the banded min distance is below that gap it
is provably the global min. The rare rows/cols that fail the certificate
(~0.3% for W=512 on gaussian clouds) are recomputed exactly in numpy.

Per band tile, the 128xW squared-distance block is produced by TensorE as
a single K=15 matmul using augmented embeddings with an fp16 hi/lo split:
    x~ = [x0,x1,x2, ||x||^2, 1],  y~ = [-2y0,-2y1,-2y2, 1, ||y||^2]
    A_n = [xh, xh, xl],  B_m = [yh, yl, yh]  (each 3x5 = K=15 rows)
(the dropped xl.yl term is ~5e-6; PSUM accumulates in fp32).

ScalarE casts PSUM->SBUF fp16. VectorE updates a column-min accumulator
(elementwise min over the overlap with previously covered columns, plain
copy for newly covered columns) and computes the per-row band min with a
2-level pairwise min tree plus one tensor_reduce. The two batches of a
core are interleaved in the free dimension of every compute tile so each
vector/scalar instruction covers both, halving instruction count.
"""

import numpy as np

import concourse.mybir as mybir
import concourse.tile as tile
from concourse import bacc
from concourse.bass_utils import run_bass_kernel_spmd

B, N, M, D = 16, 4096, 4096, 3
N_CORES = 8
BPC = B // N_CORES  # batches per core
K = 15
NT = N // 128
W = 256  # band width (y-rank window per x row-tile)
GRP = 4  # row-tiles per PSUM/cast group (GRP * BPC * W * 4B <= 8 PSUM banks)

OFFS = np.clip(128 * np.arange(NT) + 64 - W // 2, 0, M - W)

# per-column coverage in x ranks: col j is seen by tiles t with
# j in [OFFS[t], OFFS[t]+W), i.e. x rows [128t, 128t+128)
COV_LO = np.full(M, N, dtype=np.int64)
COV_HI = np.zeros(M, dtype=np.int64)
for _t in range(NT):
    _sl = slice(OFFS[_t], OFFS[_t] + W)
    COV_LO[_sl] = np.minimum(COV_LO[_sl], 128 * _t)
    COV_HI[_sl] = np.maximum(COV_HI[_sl], 128 * _t + 128)


def _strip_runs():
    """Maximal runs of 64-col strips sharing an identical covering-tile set.
    The device computes the column-min of a run directly as the elementwise
    min over the covering tiles' t16 slices (no accumulator chain)."""
    runs = []
    cur = None
    for s in range(M // 64):
        lo, hi = 64 * s, 64 * s + 64
        cov = tuple(
            t for t in range(NT) if OFFS[t] <= lo and hi <= OFFS[t] + W
        )
        assert cov, (lo, hi)
        if cur is not None and cov == cur[2]:
            cur[1] = hi
        else:
            if cur is not None:
                runs.append(tuple(cur))
            cur = [lo, hi, cov]
    runs.append(tuple(cur))
    return runs


RUNS = _strip_runs()

F16 = mybir.dt.float16
F32 = mybir.dt.float32


def _augment(xs: np.ndarray, ys: np.ndarray):
    """xs, ys: [N, 3] float64 (sorted) -> A [15, N], Bm [15, M] float16."""
    ones_x = np.ones((xs.shape[0], 1))
    ones_y = np.ones((ys.shape[0], 1))
    xt = np.concatenate([xs, (xs * xs).sum(-1, keepdims=True), ones_x], axis=-1)
    yt = np.concatenate([-2.0 * ys, ones_y, (ys * ys).sum(-1, keepdims=True)], axis=-1)
    xh = xt.astype(np.float16)
    xl = (xt - xh.astype(np.float64)).astype(np.float16)
    yh = yt.astype(np.float16)
    yl = (yt - yh.astype(np.float64)).astype(np.float16)
    A = np.concatenate([xh, xh, xl], axis=-1)  # [N, 15]
    Bm = np.concatenate([yh, yl, yh], axis=-1)
    return (
        np.ascontiguousarray(A.T).astype(np.float16),
        np.ascontiguousarray(Bm.T).astype(np.float16),
    )


def host_pack(x: np.ndarray, y: np.ndarray, return_aux: bool = False):
    """x, y: [B, N, 3] float32 -> A, B: [B, 15, N] float16 (z-sorted, lhsT/rhs
    layouts). With return_aux=True also returns the sorted fp64 coords."""
    A = np.empty((B, K, N), dtype=np.float16)
    Bm = np.empty((B, K, M), dtype=np.float16)
    xs_all = np.empty((B, N, D))
    ys_all = np.empty((B, M, D))
    for b in range(B):
        xs = x[b][np.argsort(x[b][:, 2], kind="stable")].astype(np.float64)
        ys = y[b][np.argsort(y[b][:, 2], kind="stable")].astype(np.float64)
        xs_all[b], ys_all[b] = xs, ys
        A[b], Bm[b] = _augment(xs, ys)
    if return_aux:
        return A, Bm, (xs_all, ys_all)
    return A, Bm


def build_nc(bpc: int = BPC, n: int = N, m: int = M, k: int = K, reps: int = 1):
    nt = n // 128
    nc = bacc.Bacc("TRN2", target_bir_lowering=False, debug=False)
    a_d = nc.dram_tensor("a", [bpc, k, n], F16, kind="ExternalInput")
    b_d = nc.dram_tensor("b", [bpc, k, m], F16, kind="ExternalInput")
    # per quad: the row-tree is cut after u2 ([128, GRP*bpc, W//4]); the final
    # min over W//4 runs on the host
    rm_d = nc.dram_tensor(
        "rowmins", [nt // GRP, 128, GRP * bpc, W // 4], F16, kind="ExternalOutput"
    )
    cm_d = nc.dram_tensor("colmins", [128, bpc, m], F16, kind="ExternalOutput")

    with tile.TileContext(nc) as tc:
        with (
            tc.tile_pool(name="ab", bufs=2) as ab_pool,
            tc.tile_pool(name="cast", bufs=3) as cast_pool,
            tc.tile_pool(name="acc", bufs=2) as acc_pool,
            tc.tile_pool(name="small", bufs=2) as small_pool,
            tc.tile_pool(name="scratch", bufs=2) as scratch_pool,
            tc.tile_pool(name="psum", bufs=2, space="PSUM") as psum_pool,
        ):
            for rep in range(reps):
                a_s = [
                    ab_pool.tile([k, n], F16, tag=f"a{b}", name=f"a_s{b}")
                    for b in range(bpc)
                ]
                b_s = [
                    ab_pool.tile([k, m], F16, tag=f"b{b}", name=f"b_s{b}")
                    for b in range(bpc)
                ]
                for b in range(bpc):
                    nc.sync.dma_start(a_s[b][:], a_d.ap()[b])
                    nc.sync.dma_start(b_s[b][:], b_d.ap()[b])
                cmt = acc_pool.tile([128, bpc, m], F16, tag="cmt")
                t16_of = {}  # tile index -> (t16 tile, slot base)
                run_idx = 0  # next RUNS entry to emit
                done = 0  # columns of cmt DMA'd out so far
                emitted = 0  # columns of cmt computed so far
                for p in range(nt // GRP):
                    # GRP row-tiles x bpc batches per PSUM tile; W*4B divides
                    # the 2KB bank so every matmul output stays in one bank
                    ps = psum_pool.tile([128, GRP * bpc, W], F32, tag="ps")
                    t16 = cast_pool.tile([128, GRP * bpc, W], F16, tag="t16")
                    for i in range(GRP):
                        t = GRP * p + i
                        off = int(OFFS[t])
                        for b in range(bpc):
                            nc.tensor.matmul(
                                ps[:, bpc * i + b, :],
                                a_s[b][:, t * 128 : (t + 1) * 128],
                                b_s[b][:, off : off + W],
                                start=True,
                                stop=True,
                            )
                        t16_of[t] = (t16, bpc * i)
                    # casts: one quad fully on VectorE, one split between the
                    # engines, the rest on ScalarE — balances the two engines
                    if p == 0:
                        nc.vector.tensor_copy(t16[:], ps[:])
                    elif p == 1:
                        h = GRP * bpc // 2
                        nc.scalar.copy(t16[:, :h, :], ps[:, :h, :])
                        nc.vector.tensor_copy(t16[:, h:, :], ps[:, h:, :])
                    else:
                        nc.scalar.copy(t16[:], ps[:])
                    # column-min: strip-runs whose covering tiles are all cast
                    # by now (direct min of t16 slices); steady-state chains of
                    # 128-wide 2-cover runs within this quad merge into one op
                    emit_now = []
                    while (
                        run_idx < len(RUNS)
                        and RUNS[run_idx][2][-1] <= GRP * p + GRP - 1
                    ):
                        emit_now.append(RUNS[run_idx])
                        run_idx += 1

                    def in_quad(run):
                        return (
                            len(run[2]) == 2
                            and run[1] - run[0] == 128
                            and run[2][1] == run[2][0] + 1
                            and t16_of[run[2][0]][0] is t16
                            and t16_of[run[2][1]][0] is t16
                        )

                    j = 0
                    while j < len(emit_now):
                        lo, hi, cov = emit_now[j]
                        chain = 1
                        if in_quad(emit_now[j]):
                            while j + chain < len(emit_now):
                                r2 = emit_now[j + chain]
                                if (
                                    in_quad(r2)
                                    and r2[0] == lo + 128 * chain
                                    and r2[2][0] == cov[0] + chain
                                ):
                                    chain += 1
                                else:
                                    break
                        if chain > 1:
                            s0 = t16_of[cov[0]][1]
                            in0 = t16[:, s0 : s0 + bpc * chain, W - 128 :].rearrange(
                                "p (c b) w -> p c b w", b=bpc
                            )
                            in1 = t16[
                                :, s0 + bpc : s0 + bpc * (chain + 1), 0:128
                            ].rearrange("p (c b) w -> p c b w", b=bpc)
                            dst = cmt[:, :, lo : lo + 128 * chain].rearrange(
                                "p b (c w) -> p c b w", w=128
                            )
                            nc.vector.tensor_tensor(
                                dst, in0, in1, mybir.AluOpType.min
                            )
                            emitted = lo + 128 * chain
                            j += chain
                            continue

                        def src(t):
                            tl, s = t16_of[t]
                            o = int(OFFS[t])
                            return tl[:, s : s + bpc, lo - o : hi - o]

                        dst = cmt[:, :, lo:hi]
                        if len(cov) == 1:
                            nc.vector.tensor_copy(dst, src(cov[0]))
                        else:
                            nc.vector.tensor_tensor(
                                dst, src(cov[0]), src(cov[1]), mybir.AluOpType.min
                            )
                            for tx in cov[2:]:
                                nc.vector.tensor_tensor(
                                    dst, src(tx), dst, mybir.AluOpType.min
                                )
                        emitted = hi
                        j += 1
                    # row band-min: 2-level pairwise min tree, all GRP*bpc
                    # (tile, batch) slots per op; u2 ships to the host which
                    # finishes the min over the remaining W//4 values
                    u1 = scratch_pool.tile([128, GRP * bpc, W // 2], F16, tag="u1")
                    nc.vector.tensor_tensor(
                        u1[:], t16[:, :, : W // 2], t16[:, :, W // 2 :],
                        mybir.AluOpType.min,
                    )
                    u2 = scratch_pool.tile([128, GRP * bpc, W // 4], F16, tag="u2")
                    nc.vector.tensor_tensor(
                        u2[:], u1[:, :, : W // 4], u1[:, :, W // 4 :],
                        mybir.AluOpType.min,
                    )
                    nc.sync.dma_start(rm_d.ap()[p], u2[:])
                    # stream out finalized cmt columns
                    if emitted == m or emitted - done >= 512:
                        nc.sync.dma_start(
                            cm_d.ap()[:, :, done:emitted],
                            cmt[:, :, done:emitted],
                        )
                        done = emitted
    nc.compile()
    return nc


def _finish_batch(rowmin2, colacc, xs, ys):
    """rowmin2 [N] f32 (banded row mins, squared), colacc [128, M] f16,
    xs/ys [N,3] sorted float64 -> fp32 chamfer cost for one batch
    (certificate check + exact repair)."""
    rowmin2 = np.clip(rowmin2, 0.0, None)  # [N]
    colmin2 = np.clip(colacc.astype(np.float32).min(axis=0), 0.0, None)  # [M]
    d_row = np.sqrt(rowmin2)
    d_col = np.sqrt(colmin2)
    zx, zy = xs[:, 2], ys[:, 2]

    lo = OFFS[np.arange(N) // 128]
    hi = lo + W
    left = np.where(lo > 0, zx - zy[np.maximum(lo - 1, 0)], np.inf)
    right = np.where(hi < M, zy[np.minimum(hi, M - 1)] - zx, np.inf)
    bad = np.nonzero(d_row * 1.05 + 2e-3 > np.minimum(left, right))[0]
    if bad.size:
        d_row[bad] = np.sqrt(
            ((xs[bad][:, None, :] - ys[None, :, :]) ** 2).sum(-1).min(1)
        )

    leftc = np.where(COV_LO > 0, zy - zx[np.maximum(COV_LO - 1, 0)], np.inf)
    rightc = np.where(COV_HI < N, zx[np.minimum(COV_HI, N - 1)] - zy, np.inf)
    badc = np.nonzero(d_col * 1.05 + 2e-3 > np.minimum(leftc, rightc))[0]
    if badc.size:
        d_col[badc] = np.sqrt(
            ((ys[badc][:, None, :] - xs[None, :, :]) ** 2).sum(-1).min(1)
        )

    return np.float32((d_row.mean() + d_col.mean()) * 0.5)


_RUN_KWARGS = {}
_NC_CACHE = None


def _get_nc():
    global _NC_CACHE
    if _NC_CACHE is None:
        _NC_CACHE = build_nc()
    return _NC_CACHE


def kernel(x: np.ndarray, y: np.ndarray) -> np.ndarray:
    x = np.asarray(x, dtype=np.float32)
    y = np.asarray(y, dtype=np.float32)
    A, Bm, (xs_all, ys_all) = host_pack(x, y, return_aux=True)
    nc = _get_nc()
    in_maps = [
        {"a": A[c * BPC : (c + 1) * BPC], "b": Bm[c * BPC : (c + 1) * BPC]}
        for c in range(N_CORES)
    ]
    res = run_bass_kernel_spmd(nc, in_maps, core_ids=list(range(N_CORES)), **_RUN_KWARGS)
    out = np.empty((B,), dtype=np.float32)
    for c in range(N_CORES):
        # [NT//GRP, 128, GRP*BPC, W//4] -> min over the last tree level
        rm = res.results[c]["rowmins"].astype(np.float32).min(axis=-1)
        cm = res.results[c]["colmins"]  # [128, BPC, M]
        for j in range(BPC):
            b = c * BPC + j
            # slot for (tile i, batch j) is BPC*i+j; row n = 128*(GRP*p+i)+part
            rowmin2 = rm[:, :, j::BPC].transpose(0, 2, 1).reshape(-1)
            out[b] = _finish_batch(rowmin2, cm[:, j], xs_all[b], ys_all[b])
    return out


# revision 21
# speedup vs baseline: 1.0413x; 1.0413x over previous
"""Banded chamfer distance (B=16, N=M=4096, D=3) on 8 Trainium2 NeuronCores.

Sharding: data-parallel over batch - 2 batches per core, SPMD.

Algorithm: both point clouds are sorted by the z coordinate on the host
(the output is permutation-invariant, so no un-sort is needed). After
sorting, the nearest neighbour of a point at sorted rank r is almost
always within a narrow window of ranks of the other cloud, because both
clouds are drawn from the same distribution. The device therefore only
computes a banded slice of the NxM squared-distance matrix: x row-tile t
(rows 128t..128t+127) against y columns OFFS[t]..OFFS[t]+W.

Exactness is restored on the host with a geometric certificate: for row n
the candidates excluded by the band all have |z - z_x[n]| at least the
z-gap to the band edge, so if the banded min distance is below that gap it
is provably the global min. The rare rows/cols that fail the certificate
(~0.3% for W=512 on gaussian clouds) are recomputed exactly in numpy.

Per band tile, the 128xW squared-distance block is produced by TensorE as
a single K=15 matmul using augmented embeddings with an fp16 hi/lo split:
    x~ = [x0,x1,x2, ||x||^2, 1],  y~ = [-2y0,-2y1,-2y2, 1, ||y||^2]
    A_n = [xh, xh, xl],  B_m = [yh, yl, yh]  (each 3x5 = K=15 rows)
(the dropped xl.yl term is ~5e-6; PSUM accumulates in fp32).

ScalarE casts PSUM->SBUF fp16. VectorE updates a column-min accumulator
(elementwise min over the overlap with previously covered columns, plain
copy for newly covered columns) and computes the per-row band min with a
2-level pairwise min tree plus one tensor_reduce. The two batches of a
core are interleaved in the free dimension of every compute tile so each
vector/scalar instruction covers both, halving instruction count.
"""

import numpy as np

import concourse.mybir as mybir
import concourse.tile as tile
from concourse import bacc
from concourse.bass_utils import run_bass_kernel_spmd

B, N, M, D = 16, 4096, 4096, 3
N_CORES = 8
BPC = B // N_CORES  # batches per core
K = 15
NT = N // 128
W = 256  # band width (y-rank window per x row-tile)
GRP = 4  # row-tiles per PSUM/cast group (GRP * BPC * W * 4B <= 8 PSUM banks)

OFFS = np.clip(128 * np.arange(NT) + 64 - W // 2, 0, M - W)

# per-column coverage in x ranks: col j is seen by tiles t with
# j in [OFFS[t], OFFS[t]+W), i.e. x rows [128t, 128t+128)
COV_LO = np.full(M, N, dtype=np.int64)
COV_HI = np.zeros(M, dtype=np.int64)
for _t in range(NT):
    _sl = slice(OFFS[_t], OFFS[_t] + W)
    COV_LO[_sl] = np.minimum(COV_LO[_sl], 128 * _t)
    COV_HI[_sl] = np.maximum(COV_HI[_sl], 128 * _t + 128)


def _strip_runs():
    """Maximal runs of 64-col strips sharing an identical covering-tile set.
    The device computes the column-min of a run directly as the elementwise
    min over the covering tiles' t16 slices (no accumulator chain)."""
    runs = []
    cur = None
    for s in range(M // 64):
        lo, hi = 64 * s, 64 * s + 64
        cov = tuple(
            t for t in range(NT) if OFFS[t] <= lo and hi <= OFFS[t] + W
        )
        assert cov, (lo, hi)
        if cur is not None and cov == cur[2]:
            cur[1] = hi
        else:
            if cur is not None:
                runs.append(tuple(cur))
            cur = [lo, hi, cov]
    runs.append(tuple(cur))
    return runs


RUNS = _strip_runs()

F16 = mybir.dt.float16
F32 = mybir.dt.float32


def _augment(xs: np.ndarray, ys: np.ndarray):
    """xs, ys: [N, 3] float64 (sorted) -> A [15, N], Bm [15, M] float16."""
    ones_x = np.ones((xs.shape[0], 1))
    ones_y = np.ones((ys.shape[0], 1))
    xt = np.concatenate([xs, (xs * xs).sum(-1, keepdims=True), ones_x], axis=-1)
    yt = np.concatenate([-2.0 * ys, ones_y, (ys * ys).sum(-1, keepdims=True)], axis=-1)
    xh = xt.astype(np.float16)
    xl = (xt - xh.astype(np.float64)).astype(np.float16)
    yh = yt.astype(np.float16)
    yl = (yt - yh.astype(np.float64)).astype(np.float16)
    A = np.concatenate([xh, xh, xl], axis=-1)  # [N, 15]
    Bm = np.concatenate([yh, yl, yh], axis=-1)
    return (
        np.ascontiguousarray(A.T).astype(np.float16),
        np.ascontiguousarray(Bm.T).astype(np.float16),
    )


def host_pack(x: np.ndarray, y: np.ndarray, return_aux: bool = False):
    """x, y: [B, N, 3] float32 -> A, B: [B, 15, N] float16 (z-sorted, lhsT/rhs
    layouts). With return_aux=True also returns the sorted fp64 coords."""
    A = np.empty((B, K, N), dtype=np.float16)
    Bm = np.empty((B, K, M), dtype=np.float16)
    xs_all = np.empty((B, N, D))
    ys_all = np.empty((B, M, D))
    for b in range(B):
        xs = x[b][np.argsort(x[b][:, 2], kind="stable")].astype(np.float64)
        ys = y[b][np.argsort(y[b][:, 2], kind="stable")].astype(np.float64)
        xs_all[b], ys_all[b] = xs, ys
        A[b], Bm[b] = _augment(xs, ys)
    if return_aux:
        return A, Bm, (xs_all, ys_all)
    return A, Bm


def build_nc(bpc: int = BPC, n: int = N, m: int = M, k: int = K, reps: int = 1):
    nt = n // 128
    nc = bacc.Bacc("TRN2", target_bir_lowering=False, debug=False)
    a_d = nc.dram_tensor("a", [bpc, k, n], F16, kind="ExternalInput")
    b_d = nc.dram_tensor("b", [bpc, k, m], F16, kind="ExternalInput")
    # per quad: the row-tree is cut after u2 ([128, GRP*bpc, W//4]); the final
    # min over W//4 runs on the host
    rm_d = nc.dram_tensor(
        "rowmins", [nt // GRP, 128, GRP * bpc, W // 4], F16, kind="ExternalOutput"
    )
    cm_d = nc.dram_tensor("colmins", [128, bpc, m], F16, kind="ExternalOutput")

    with tile.TileContext(nc) as tc:
        with (
            tc.tile_pool(name="ab", bufs=2) as ab_pool,
            tc.tile_pool(name="cast", bufs=3) as cast_pool,
            tc.tile_pool(name="acc", bufs=2) as acc_pool,
            tc.tile_pool(name="small", bufs=2) as small_pool,
            tc.tile_pool(name="scratch", bufs=2) as scratch_pool,
            tc.tile_pool(name="psum", bufs=2, space="PSUM") as psum_pool,
        ):
            for rep in range(reps):
                a_s = [
                    ab_pool.tile([k, n], F16, tag=f"a{b}", name=f"a_s{b}")
                    for b in range(bpc)
                ]
                b_s = [
                    ab_pool.tile([k, m], F16, tag=f"b{b}", name=f"b_s{b}")
                    for b in range(bpc)
                ]
                for b in range(bpc):
                    nc.sync.dma_start(a_s[b][:], a_d.ap()[b])
                    nc.sync.dma_start(b_s[b][:], b_d.ap()[b])
                cmt = acc_pool.tile([128, bpc, m], F16, tag="cmt")
                t16_of = {}  # tile index -> (t16 tile, slot base)
                run_idx = 0  # next RUNS entry to emit
                done = 0  # columns of cmt DMA'd out so far
                emitted = 0  # columns of cmt computed so far
                for p in range(nt // GRP):
                    # GRP row-tiles x bpc batches per PSUM tile; W*4B divides
                    # the 2KB bank so every matmul output stays in one bank
                    ps = psum_pool.tile([128, GRP * bpc, W], F32, tag="ps")
                    t16 = cast_pool.tile([128, GRP * bpc, W], F16, tag="t16")
                    for i in range(GRP):
                        t = GRP * p + i
                        off = int(OFFS[t])
                        for b in range(bpc):
                            nc.tensor.matmul(
                                ps[:, bpc * i + b, :],
                                a_s[b][:, t * 128 : (t + 1) * 128],
                                b_s[b][:, off : off + W],
                                start=True,
                                stop=True,
                            )
                        t16_of[t] = (t16, bpc * i)
                    # one quad's cast runs on VectorE to balance the engines
                    if p == 0:
                        nc.vector.tensor_copy(t16[:], ps[:])
                    else:
                        nc.scalar.copy(t16[:], ps[:])
                    # column-min: one op per strip-run whose covering tiles
                    # are all cast by now (direct min of t16 slices)
                    while run_idx < len(RUNS) and RUNS[run_idx][2][-1] <= GRP * p + GRP - 1:
                        lo, hi, cov = RUNS[run_idx]
                        run_idx += 1

                        def src(t):
                            tl, s = t16_of[t]
                            o = int(OFFS[t])
                            return tl[:, s : s + bpc, lo - o : hi - o]

                        dst = cmt[:, :, lo:hi]
                        if len(cov) == 1:
                            nc.vector.tensor_copy(dst, src(cov[0]))
                        else:
                            nc.vector.tensor_tensor(
                                dst, src(cov[0]), src(cov[1]), mybir.AluOpType.min
                            )
                            for tx in cov[2:]:
                                nc.vector.tensor_tensor(
                                    dst, src(tx), dst, mybir.AluOpType.min
                                )
                        emitted = hi
                    # row band-min: 2-level pairwise min tree, all GRP*bpc
                    # (tile, batch) slots per op; u2 ships to the host which
                    # finishes the min over the remaining W//4 values
                    u1 = scratch_pool.tile([128, GRP * bpc, W // 2], F16, tag="u1")
                    nc.vector.tensor_tensor(
                        u1[:], t16[:, :, : W // 2], t16[:, :, W // 2 :],
                        mybir.AluOpType.min,
                    )
                    u2 = scratch_pool.tile([128, GRP * bpc, W // 4], F16, tag="u2")
                    nc.vector.tensor_tensor(
                        u2[:], u1[:, :, : W // 4], u1[:, :, W // 4 :],
                        mybir.AluOpType.min,
                    )
                    nc.sync.dma_start(rm_d.ap()[p], u2[:])
                    # stream out finalized cmt columns
                    if emitted == m or emitted - done >= 512:
                        nc.sync.dma_start(
                            cm_d.ap()[:, :, done:emitted],
                            cmt[:, :, done:emitted],
                        )
                        done = emitted
    nc.compile()
    return nc


def _finish_batch(rowmin2, colacc, xs, ys):
    """rowmin2 [N] f32 (banded row mins, squared), colacc [128, M] f16,
    xs/ys [N,3] sorted float64 -> fp32 chamfer cost for one batch
    (certificate check + exact repair)."""
    rowmin2 = np.clip(rowmin2, 0.0, None)  # [N]
    colmin2 = np.clip(colacc.astype(np.float32).min(axis=0), 0.0, None)  # [M]
    d_row = np.sqrt(rowmin2)
    d_col = np.sqrt(colmin2)
    zx, zy = xs[:, 2], ys[:, 2]

    lo = OFFS[np.arange(N) // 128]
    hi = lo + W
    left = np.where(lo > 0, zx - zy[np.maximum(lo - 1, 0)], np.inf)
    right = np.where(hi < M, zy[np.minimum(hi, M - 1)] - zx, np.inf)
    bad = np.nonzero(d_row * 1.05 + 2e-3 > np.minimum(left, right))[0]
    if bad.size:
        d_row[bad] = np.sqrt(
            ((xs[bad][:, None, :] - ys[None, :, :]) ** 2).sum(-1).min(1)
        )

    leftc = np.where(COV_LO > 0, zy - zx[np.maximum(COV_LO - 1, 0)], np.inf)
    rightc = np.where(COV_HI < N, zx[np.minimum(COV_HI, N - 1)] - zy, np.inf)
    badc = np.nonzero(d_col * 1.05 + 2e-3 > np.minimum(leftc, rightc))[0]
    if badc.size:
        d_col[badc] = np.sqrt(
            ((ys[badc][:, None, :] - xs[None, :, :]) ** 2).sum(-1).min(1)
        )

    return np.float32((d_row.mean() + d_col.mean()) * 0.5)


_RUN_KWARGS = {}
_NC_CACHE = None


def _get_nc():
    global _NC_CACHE
    if _NC_CACHE is None:
        _NC_CACHE = build_nc()
    return _NC_CACHE


def kernel(x: np.ndarray, y: np.ndarray) -> np.ndarray:
    x = np.asarray(x, dtype=np.float32)
    y = np.asarray(y, dtype=np.float32)
    A, Bm, (xs_all, ys_all) = host_pack(x, y, return_aux=True)
    nc = _get_nc()
    in_maps = [
        {"a": A[c * BPC : (c + 1) * BPC], "b": Bm[c * BPC : (c + 1) * BPC]}
        for c in range(N_CORES)
    ]
    res = run_bass_kernel_spmd(nc, in_maps, core_ids=list(range(N_CORES)), **_RUN_KWARGS)
    out = np.empty((B,), dtype=np.float32)
    for c in range(N_CORES):
        # [NT//GRP, 128, GRP*BPC, W//4] -> min over the last tree level
        rm = res.results[c]["rowmins"].astype(np.float32).min(axis=-1)
        cm = res.results[c]["colmins"]  # [128, BPC, M]
        for j in range(BPC):
            b = c * BPC + j
            # slot for (tile i, batch j) is BPC*i+j; row n = 128*(GRP*p+i)+part
            rowmin2 = rm[:, :, j::BPC].transpose(0, 2, 1).reshape(-1)
            out[b] = _finish_batch(rowmin2, cm[:, j], xs_all[b], ys_all[b])
    return out
